# revision 18
# baseline (speedup 1.0000x reference)
"""Additive-attention Bass kernel for Trainium2, data-parallel over batch on 8 cores.

Math per batch b:
    q = queries[b] @ W_q                      # (H,)
    kp[t, h] = sum_d keys[b, t, d] W_k[d, h]  # (Tk, H)
    feat = tanh(q + kp)                       # (Tk, H)
    s[t] = feat[t] . w_v                      # (Tk,)
    attn = softmax(s)                         # = exp(s) / sum exp(s)  (no max-sub
                                              #   needed: |s| <= ||w_v||_1 ~ 13)
    out[b] = attn @ values[b]                 # (B, V)

v2: keys are pre-transposed AND pre-tiled on the host into the exact per-load
SBUF image, so the on-chip transpose stage (PE pass-through transposes + DVE
f32->f16 casts + kT PSUM->SBUF copies) disappears entirely. KP consumes the
raw f32 keys as float32r (~1.3 cycles/row at free-dim 512). PE work per 512-t
chunk drops from ~5.4us to ~4.5us and DVE to ~0, under the per-chunk DMA
budget: the kernel is DMA-bound wall to wall.

The on-chip t-axis is PERMUTED, inherited from the p-major values tile layout
("(p n) d"): within a load, chunk-position cc*512 + j*128 + q holds the token
t = q*16 + cc*4 + j. Keys are packed on the host in the same permutation so
the flush pairs each attention weight with its own value row; softmax and Z
are permutation-invariant so nothing else cares.

v2.1: one keys DMA and one values DMA per (pair, load) covering BOTH parities
(4MB each, 32KB per partition contiguous, 128 descriptors) - halving the
dynamic-DMA descriptor-ring refill traffic that rides engine 0's queue XIV and
was the critical-path straggler. Weights/queries/w_v ride the ACT HWDGE queue
so the 64MB stream starts on the sync queue at t~0 while setup overlaps it.

TWO batches (parities) are interleaved per chunk. KP matmuls are ordered
stationary-major (ht, dt) with parities inner so each W_k 128x128 slice is
loaded once per chunk (4 LDWs instead of 8). The score stage S runs on chunk
c-1 (giving tanh a full chunk of runway) and the values flush V on chunk c-2.
Z partials accumulate in ACT during exp; each parity's output row accumulates
in its own PSUM bank (a group-start marks the whole 2KB bank pending-zero, so
interleaved accumulation groups must not share one).
"""

import numpy as np

import concourse.bass as bass
import concourse.mybir as mybir
import concourse.tile as tile
from concourse import bacc
from concourse.bass import ts
from concourse.bass_utils import run_bass_kernel_spmd
from concourse.masks import make_identity

B, TK, D, H = 32, 8192, 256, 256
NCORES = 8
BL = B // NCORES          # batches per core
NPAIR = BL // 2
CHUNK = 512               # t-chunk per compute iteration
NCHUNK = TK // CHUNK
NSUB = CHUNK // 128
TT = 2048                 # t-span per DMA load
NL = TK // TT             # loads per batch
NCC = TT // CHUNK         # compute chunks per load
NNT = TT // 128           # n-slices per load tile

F32 = mybir.dt.float32
F32R = mybir.dt.float32r
F16 = mybir.dt.float16
AF = mybir.ActivationFunctionType


def build():
    nc = bacc.Bacc("TRN2", target_bir_lowering=False, debug=False, num_devices=NCORES)
    # keys/values arrive host-packed per (pair, load): [p, par, ...] with 32KB
    # contiguous per partition (see module docstring)
    keys_d = nc.dram_tensor(
        "keys", [NPAIR, NL, 128, 2, 2, TT], F32R, kind="ExternalInput"
    ).ap()
    vals_d = nc.dram_tensor(
        "values", [NPAIR, NL, 2, 128, NNT, D], F32R, kind="ExternalInput"
    ).ap()
    qrs_d = nc.dram_tensor("queries", [BL, D], F32, kind="ExternalInput").ap()
    wq_d = nc.dram_tensor("W_q", [D, H], F32, kind="ExternalInput").ap()
    wk_d = nc.dram_tensor("W_k", [D, H], F32R, kind="ExternalInput").ap()
    wv_d = nc.dram_tensor("w_v", [1, H], F32, kind="ExternalInput").ap()
    out_d = nc.dram_tensor("out", [BL, D], F32, kind="ExternalOutput").ap()

    with tile.TileContext(nc) as tc:
        with (
            tc.tile_pool(name="consts", bufs=1) as consts,
            tc.tile_pool(name="kin", bufs=2) as kin,
            tc.tile_pool(name="vin", bufs=2) as vin,
            tc.tile_pool(name="mid", bufs=2) as mid,
            tc.tile_pool(name="small", bufs=2) as small,
        ):
            kf_hist = {}  # (pair, L) -> keys tile [128, par, dt, TT]
            va_hist = {}  # (pair, L, par) -> vals tile [128, n, D]

            def issue_dma_for(pr, L):
                # keys first (needed by KP at chunk cc=0), one 4MB DMA for both
                # parities (32KB/partition contiguous -> 128 descriptors);
                # values aren't read until the lag-2 flush later
                kf = kin.tile([128, 2, 2, TT], F32R, tag="keys", name="kf")
                nc.sync.dma_start(out=kf, in_=keys_d[pr, L])
                kf_hist[(pr, L)] = kf
                for par in range(2):
                    va = vin.tile([128, NNT, D], F32R, tag=f"vals{par}", name="va", bufs=3)
                    nc.sync.dma_start(out=va, in_=vals_d[pr, L, par])
                    va_hist[(pr, L, par)] = va

            ident_f32 = consts.tile([128, 128], F32)
            make_identity(nc, ident_f32)
            one11 = consts.tile([1, 1], F32)
            nc.vector.memset(one11, 1.0)
            ones_col = consts.tile([128, 1], F32)
            nc.vector.memset(ones_col, 1.0)
            negc = consts.tile([128, 1], F32)
            nc.vector.memset(negc, -6.0)

            wk_s = consts.tile([128, 2, H], F32R)
            nc.sync.dma_start(out=wk_s, in_=wk_d.rearrange("(dt p) h -> p dt h", p=128))
            wq_s = consts.tile([128, 2, H], F32)
            nc.sync.dma_start(out=wq_s, in_=wq_d.rearrange("(dt p) h -> p dt h", p=128))
            wv_row = consts.tile([1, H], F32)
            nc.sync.dma_start(out=wv_row, in_=wv_d)
            q_rows = consts.tile([1, BL * D], F32)
            nc.sync.dma_start(
                out=q_rows, in_=qrs_d.rearrange("b d -> (b d)").rearrange("(o f) -> o f", o=1)
            )

            issue_dma_for(0, 0)

            wv_cols = consts.tile([128, 2], F16)      # w_v as [h, htile] columns
            q_cols = consts.tile([128, BL, 2], F32)  # q biases [h, b, htile]

            # ---- setup: w_v columns and per-batch q biases (all tiny) ----
            with tc.tile_pool(name="setup_ps", bufs=1, space="PSUM") as setup_ps:
                ps_wv = setup_ps.tile([128, 2], F32)
                for ht in range(2):
                    nc.tensor.matmul(
                        out=ps_wv[:, ht : ht + 1],
                        lhsT=wv_row[0:1, ts(ht, 128)],
                        rhs=one11,
                        is_transpose=True,
                    )
                nc.vector.tensor_copy(out=wv_cols, in_=ps_wv)

                for b in range(BL):
                    ps_qc = setup_ps.tile([128, 2], F32, tag="ps_qc")
                    for dt in range(2):
                        nc.tensor.matmul(
                            out=ps_qc[:, dt : dt + 1],
                            lhsT=q_rows[0:1, b * D + dt * 128 : b * D + (dt + 1) * 128],
                            rhs=one11,
                            is_transpose=True,
                        )
                    qc_s = small.tile([128, 2], F32, tag="qc_s")
                    nc.vector.tensor_copy(out=qc_s, in_=ps_qc)
                    ps_q = setup_ps.tile([128, 2], F32, tag="ps_q")
                    for ht in range(2):
                        for dt in range(2):
                            nc.tensor.matmul(
                                out=ps_q[:, ht : ht + 1],
                                lhsT=wq_s[:, dt, ts(ht, 128)],
                                rhs=qc_s[:, dt : dt + 1],
                                start=(dt == 0),
                                stop=(dt == 1),
                            )
                    nc.vector.tensor_copy(out=q_cols[:, b, :], in_=ps_q)

            # ---- main loop: two batches (parities) interleaved per pair ----
            with (
                tc.tile_pool(name="pkp", bufs=1, space="PSUM") as pkpp,
                tc.tile_pool(name="scol", bufs=2, space="PSUM") as scolp,
                tc.tile_pool(name="pout", bufs=1, space="PSUM") as poutp,
            ):
                for pair in range(NPAIR):
                    bs = (2 * pair, 2 * pair + 1)
                    # separate PSUM bank per parity: a group-start marks the
                    # whole 2KB bank pending-zero, so two interleaved
                    # accumulation groups must not share a bank
                    psum_outs = [
                        poutp.tile([1, D], F32, tag=f"po{par}", name=f"po{par}")
                        for par in range(2)
                    ]
                    z_pps = [
                        small.tile([128, NCHUNK], F32, tag=f"zpp{par}", name=f"zpp{par}")
                        for par in range(2)
                    ]
                    pends = [[], []]   # awaiting values flush: (ec, vals, c)
                    sq = [[], []]      # awaiting score stage: (feat, vals, c)

                    def flush_pend(par, last):
                        ec_p, vals_p, c_p = pends[par].pop(0)
                        cc_p = c_p % NCC
                        for j in range(NSUB):
                            nc.tensor.matmul(
                                out=psum_outs[par],
                                lhsT=ec_p[:, j : j + 1],
                                rhs=vals_p[:, cc_p * NSUB + j, :],
                                start=(c_p == 0 and j == 0),
                                stop=(last and j == NSUB - 1),
                                skip_group_check=True,
                            )

                    def do_scores(par):
                        # score stage for the OLDEST queued chunk (lag-1: its
                        # tanh has had a full chunk-pair to finish)
                        fe, vt, cp = sq[par].pop(0)
                        scol = scolp.tile([128, NSUB], F32, tag="scol", name="scol")
                        for j in range(NSUB):
                            for ht in range(2):
                                nc.tensor.matmul(
                                    out=scol[:, j : j + 1],
                                    lhsT=fe[:, ht, ts(j, 128)],
                                    rhs=wv_cols[:, ht : ht + 1],
                                    start=(ht == 0),
                                    stop=(ht == 1),
                                )
                        ec = small.tile([128, NSUB], F32R, tag=f"ec{par}", bufs=3)
                        nc.scalar.activation(
                            out=ec,
                            in_=scol,
                            func=AF.Exp,
                            bias=negc[:, 0:1],
                            accum_out=z_pps[par][:, cp : cp + 1],
                        )
                        pends[par].append((ec, vt, cp))

                    for L in range(NL):
                        keys_cur = kf_hist[(pair, L)]
                        vals_cur = [va_hist[(pair, L, 0)], va_hist[(pair, L, 1)]]
                        if L + 1 < NL:
                            issue_dma_for(pair, L + 1)
                        elif pair + 1 < NPAIR:
                            # hoist the NEXT pair's first loads ahead of this
                            # pair's tail so the DMA queue never sits behind
                            # the compute-dependent normalization/stores
                            issue_dma_for(pair + 1, 0)
                        for cc in range(NCC):
                            c = L * NCC + cc

                            # ---- KP for both parities, stationary-major so
                            # each W_k 128x128 slice loads once per chunk.
                            # kp banks interleave accumulation groups, hence
                            # skip_group_check.
                            kps = [
                                [
                                    pkpp.tile(
                                        [128, CHUNK], F32,
                                        tag=f"kp{par}{ht}", name=f"kp{par}{ht}",
                                    )
                                    for ht in range(2)
                                ]
                                for par in range(2)
                            ]
                            for ht in range(2):
                                for dt in range(2):
                                    for par in range(2):
                                        nc.tensor.matmul(
                                            out=kps[par][ht],
                                            lhsT=wk_s[:, dt, ts(ht, 128)],
                                            rhs=keys_cur[:, par, dt, ts(cc, CHUNK)],
                                            start=(dt == 0),
                                            stop=(dt == 1),
                                            skip_group_check=True,
                                        )

                            # ---- per parity: tanh, then lag-2 values flush
                            for par in range(2):
                                feat = mid.tile([128, 2, CHUNK], F16, tag=f"feat{par}")
                                for ht in range(2):
                                    nc.scalar.activation(
                                        out=feat[:, ht, :],
                                        in_=kps[par][ht],
                                        func=AF.Tanh,
                                        bias=q_cols[:, bs[par], ht : ht + 1],
                                        scale=1.0,
                                    )
                                sq[par].append((feat, vals_cur[par], c))
                                if pends[par]:
                                    flush_pend(par, last=False)

                            # ---- S: score columns, exp, Z for chunk c-1
                            for par in range(2):
                                if len(sq[par]) >= 2:
                                    do_scores(par)

                    # ---- tail: drain score + flush stages, normalize, store ----
                    for par in range(2):
                        do_scores(par)
                    for par in range(2):
                        flush_pend(par, last=False)
                        flush_pend(par, last=True)
                    for par in range(2):
                        b = bs[par]
                        # Z = sum over partitions and chunks of z_pp: one
                        # matmul ones^T @ z_pps -> [1,16] column sums on
                        # partition 0, then a free-axis reduce.
                        zrow_ps = scolp.tile([1, NCHUNK], F32, tag="scol", name=f"zr{par}")
                        nc.tensor.matmul(
                            out=zrow_ps, lhsT=ones_col, rhs=z_pps[par]
                        )
                        z1 = small.tile([1, 1], F32, tag=f"z{par}")
                        nc.vector.reduce_sum(
                            out=z1, in_=zrow_ps, axis=mybir.AxisListType.X
                        )
                        rz = small.tile([1, 1], F32, tag=f"rz{par}")
                        nc.vector.reciprocal(out=rz, in_=z1)
                        orow = small.tile([1, D], F32, tag=f"orow{par}")
                        nc.scalar.mul(
                            out=orow, in_=psum_outs[par], mul=rz[0:1, 0:1]
                        )
                        nc.sync.dma_start(out=out_d[b : b + 1, :], in_=orow)

    nc.compile()
    return nc


_NC = None


def _get_nc():
    global _NC
    if _NC is None:
        _NC = build()
    return _NC


def _pack_keys(kcore):
    """[BL, TK, D] f32 -> [NPAIR, NL, 128, 2(par), 2(dt), TT]: the per-load SBUF
    image (both parities, 32KB contiguous per partition), with the chip-side
    t-permutation that matches the values tile layout.

    Values load p-major: va[q, n] holds t = L*TT + q*16 + n with n = cc*4 + j.
    The flush for (cc, j) contracts ec[q, j] against va[:, cc*4+j], so the
    score pipeline must emit chunk-position j*128+q <-> that same t. Scores
    inherit the keys free axis, hence keys position t' = cc*512+j*128+q must
    hold t = q*16 + cc*4 + j:
        kf[p, dt, cc*512 + j*128 + q] = keys[b, L*TT + q*16 + cc*4 + j, dt*128+p]
    """
    a = kcore.reshape(NPAIR, 2, NL, 128, NCC, NSUB, 2, 128)
    # axes: (pair, par, L, q, cc, j, dt, p) -> (pair, L, p, par, dt, cc, j, q)
    a = a.transpose(0, 2, 7, 1, 6, 4, 5, 3)
    return np.ascontiguousarray(a.reshape(NPAIR, NL, 128, 2, 2, TT))


def _pack_vals(vcore):
    """[BL, TK, D] f32 -> [NPAIR, NL, 2(par), 128, NNT, D]: the p-major values
    tile image vv[p, n] = values[2*pair+par, L*TT + p*16 + n]."""
    a = vcore.reshape(NPAIR, 2, NL, 128, NNT, D)
    a = a.transpose(0, 2, 1, 3, 4, 5)
    return np.ascontiguousarray(a)


def make_in_maps(queries, keys, values, W_q, W_k, w_v):
    queries = np.asarray(queries, np.float32)
    keys = np.asarray(keys, np.float32)
    values = np.asarray(values, np.float32)
    W_q = np.ascontiguousarray(np.asarray(W_q, np.float32))
    W_k = np.ascontiguousarray(np.asarray(W_k, np.float32))
    wv2 = np.ascontiguousarray(np.asarray(w_v, np.float32).reshape(1, H))
    in_maps = []
    for i in range(NCORES):
        sl = slice(i * BL, (i + 1) * BL)
        in_maps.append(
            {
                "queries": np.ascontiguousarray(queries[sl]),
                "keys": _pack_keys(keys[sl]),
                "values": _pack_vals(values[sl]),
                "W_q": W_q,
                "W_k": W_k,
                "w_v": wv2,
            }
        )
    return in_maps


def kernel(queries, keys, values, W_q, W_k, w_v):
    nc = _get_nc()
    in_maps = make_in_maps(queries, keys, values, W_q, W_k, w_v)
    res = run_bass_kernel_spmd(nc, in_maps, list(range(NCORES)))
    return np.concatenate([res.results[i]["out"] for i in range(NCORES)], axis=0)


# revision 19
# speedup vs baseline: 1.0166x; 1.0166x over previous
"""Additive-attention Bass kernel for Trainium2, data-parallel over batch on 8 cores.

Math per batch b:
    q = queries[b] @ W_q                      # (H,)
    kp[t, h] = sum_d keys[b, t, d] W_k[d, h]  # (Tk, H)
    feat = tanh(q + kp)                       # (Tk, H)
    s[t] = feat[t] . w_v                      # (Tk,)
    attn = softmax(s)                         # = exp(s) / sum exp(s)  (no max-sub
                                              #   needed: |s| <= ||w_v||_1 ~ 13)
    out[b] = attn @ values[b]                 # (B, V)

v2: keys are pre-transposed AND pre-tiled on the host into the exact per-load
SBUF image, so the on-chip transpose stage (PE pass-through transposes + DVE
f32->f16 casts + kT PSUM->SBUF copies) disappears entirely. KP consumes the
raw f32 keys as float32r (~1.3 cycles/row at free-dim 512). PE work per 512-t
chunk drops from ~5.4us to ~4.5us and DVE to ~0, under the per-chunk DMA
budget: the kernel is DMA-bound wall to wall.

The on-chip t-axis is PERMUTED, inherited from the p-major values tile layout
("(p n) d"): within a load, chunk-position cc*512 + j*128 + q holds the token
t = q*16 + cc*4 + j. Keys are packed on the host in the same permutation so
the flush pairs each attention weight with its own value row; softmax and Z
are permutation-invariant so nothing else cares.

v2.1: one keys DMA and one values DMA per (pair, load) covering BOTH parities
(4MB each, 32KB per partition contiguous, 128 descriptors) - halving the
dynamic-DMA descriptor-ring refill traffic that rides engine 0's queue XIV and
was the critical-path straggler. Weights/queries/w_v ride the ACT HWDGE queue
so the 64MB stream starts on the sync queue at t~0 while setup overlaps it.

TWO batches (parities) are interleaved per chunk. KP matmuls are ordered
stationary-major (ht, dt) with parities inner so each W_k 128x128 slice is
loaded once per chunk (4 LDWs instead of 8). The score stage S runs on chunk
c-1 (giving tanh a full chunk of runway) and the values flush V on chunk c-2.
Z partials accumulate in ACT during exp; each parity's output row accumulates
in its own PSUM bank (a group-start marks the whole 2KB bank pending-zero, so
interleaved accumulation groups must not share one).
"""

import numpy as np

import concourse.bass as bass
import concourse.mybir as mybir
import concourse.tile as tile
from concourse import bacc
from concourse.bass import ts
from concourse.bass_utils import run_bass_kernel_spmd
from concourse.masks import make_identity

B, TK, D, H = 32, 8192, 256, 256
NCORES = 8
BL = B // NCORES          # batches per core
NPAIR = BL // 2
CHUNK = 512               # t-chunk per compute iteration
NCHUNK = TK // CHUNK
NSUB = CHUNK // 128
TT = 2048                 # t-span per DMA load
NL = TK // TT             # loads per batch
NCC = TT // CHUNK         # compute chunks per load
NNT = TT // 128           # n-slices per load tile

F32 = mybir.dt.float32
F32R = mybir.dt.float32r
F16 = mybir.dt.float16
AF = mybir.ActivationFunctionType


def build():
    nc = bacc.Bacc("TRN2", target_bir_lowering=False, debug=False, num_devices=NCORES)
    # keys/values arrive host-packed per (pair, load): [p, par, ...] with 32KB
    # contiguous per partition (see module docstring)
    keys_d = nc.dram_tensor(
        "keys", [NPAIR, NL, 2, 128, 2, TT], F32R, kind="ExternalInput"
    ).ap()
    vals_d = nc.dram_tensor(
        "values", [NPAIR, NL, 2, 128, NNT, D], F32R, kind="ExternalInput"
    ).ap()
    qrs_d = nc.dram_tensor("queries", [BL, D], F32, kind="ExternalInput").ap()
    wq_d = nc.dram_tensor("W_q", [D, H], F32, kind="ExternalInput").ap()
    wk_d = nc.dram_tensor("W_k", [D, H], F32R, kind="ExternalInput").ap()
    wv_d = nc.dram_tensor("w_v", [1, H], F32, kind="ExternalInput").ap()
    out_d = nc.dram_tensor("out", [BL, D], F32, kind="ExternalOutput").ap()

    with tile.TileContext(nc) as tc:
        with (
            tc.tile_pool(name="consts", bufs=1) as consts,
            tc.tile_pool(name="kin", bufs=2) as kin,
            tc.tile_pool(name="vin", bufs=2) as vin,
            tc.tile_pool(name="mid", bufs=2) as mid,
            tc.tile_pool(name="small", bufs=2) as small,
        ):
            kf_hist = {}  # (pair, L, par) -> keys tile [128, dt, TT]
            va_hist = {}  # (pair, L, par) -> vals tile [128, n, D]

            def issue_dma_for(pr, L):
                # keys first (needed by KP at chunk cc=0); values aren't read
                # until the lag-2 flush later
                for par in range(2):
                    kf = kin.tile([128, 2, TT], F32R, tag=f"keys{par}", name="kf")
                    nc.sync.dma_start(out=kf, in_=keys_d[pr, L, par])
                    kf_hist[(pr, L, par)] = kf
                for par in range(2):
                    va = vin.tile([128, NNT, D], F32R, tag=f"vals{par}", name="va", bufs=3)
                    nc.sync.dma_start(out=va, in_=vals_d[pr, L, par])
                    va_hist[(pr, L, par)] = va

            # keys p0 of the very first load goes FIRST on the sync queue so
            # the 64MB stream starts immediately; the tiny consts follow it,
            # then the rest of load 0.
            kf0 = kin.tile([128, 2, TT], F32R, tag="keys0", name="kf0")
            nc.sync.dma_start(out=kf0, in_=keys_d[0, 0, 0])
            kf_hist[(0, 0, 0)] = kf0

            ident_f32 = consts.tile([128, 128], F32)
            make_identity(nc, ident_f32)
            one11 = consts.tile([1, 1], F32)
            nc.vector.memset(one11, 1.0)
            ones_col = consts.tile([128, 1], F32)
            nc.vector.memset(ones_col, 1.0)
            negc = consts.tile([128, 1], F32)
            nc.vector.memset(negc, -6.0)

            wk_s = consts.tile([128, 2, H], F32R)
            nc.sync.dma_start(out=wk_s, in_=wk_d.rearrange("(dt p) h -> p dt h", p=128))
            wq_s = consts.tile([128, 2, H], F32)
            nc.sync.dma_start(out=wq_s, in_=wq_d.rearrange("(dt p) h -> p dt h", p=128))
            wv_row = consts.tile([1, H], F32)
            nc.sync.dma_start(out=wv_row, in_=wv_d)
            q_rows = consts.tile([1, BL * D], F32)
            nc.sync.dma_start(
                out=q_rows, in_=qrs_d.rearrange("b d -> (b d)").rearrange("(o f) -> o f", o=1)
            )

            kf1 = kin.tile([128, 2, TT], F32R, tag="keys1", name="kf1")
            nc.sync.dma_start(out=kf1, in_=keys_d[0, 0, 1])
            kf_hist[(0, 0, 1)] = kf1
            for par in range(2):
                va0 = vin.tile([128, NNT, D], F32R, tag=f"vals{par}", name="va0", bufs=3)
                nc.sync.dma_start(out=va0, in_=vals_d[0, 0, par])
                va_hist[(0, 0, par)] = va0

            wv_cols = consts.tile([128, 2], F16)      # w_v as [h, htile] columns
            q_cols = consts.tile([128, BL, 2], F32)  # q biases [h, b, htile]

            # ---- setup: w_v columns and per-batch q biases (all tiny) ----
            with tc.tile_pool(name="setup_ps", bufs=1, space="PSUM") as setup_ps:
                ps_wv = setup_ps.tile([128, 2], F32)
                for ht in range(2):
                    nc.tensor.matmul(
                        out=ps_wv[:, ht : ht + 1],
                        lhsT=wv_row[0:1, ts(ht, 128)],
                        rhs=one11,
                        is_transpose=True,
                    )
                nc.vector.tensor_copy(out=wv_cols, in_=ps_wv)

                for b in range(BL):
                    ps_qc = setup_ps.tile([128, 2], F32, tag="ps_qc")
                    for dt in range(2):
                        nc.tensor.matmul(
                            out=ps_qc[:, dt : dt + 1],
                            lhsT=q_rows[0:1, b * D + dt * 128 : b * D + (dt + 1) * 128],
                            rhs=one11,
                            is_transpose=True,
                        )
                    qc_s = small.tile([128, 2], F32, tag="qc_s")
                    nc.vector.tensor_copy(out=qc_s, in_=ps_qc)
                    ps_q = setup_ps.tile([128, 2], F32, tag="ps_q")
                    for ht in range(2):
                        for dt in range(2):
                            nc.tensor.matmul(
                                out=ps_q[:, ht : ht + 1],
                                lhsT=wq_s[:, dt, ts(ht, 128)],
                                rhs=qc_s[:, dt : dt + 1],
                                start=(dt == 0),
                                stop=(dt == 1),
                            )
                    nc.vector.tensor_copy(out=q_cols[:, b, :], in_=ps_q)

            # ---- main loop: two batches (parities) interleaved per pair ----
            with (
                tc.tile_pool(name="pkp", bufs=1, space="PSUM") as pkpp,
                tc.tile_pool(name="scol", bufs=2, space="PSUM") as scolp,
                tc.tile_pool(name="pout", bufs=1, space="PSUM") as poutp,
            ):
                for pair in range(NPAIR):
                    bs = (2 * pair, 2 * pair + 1)
                    # separate PSUM bank per parity: a group-start marks the
                    # whole 2KB bank pending-zero, so two interleaved
                    # accumulation groups must not share a bank
                    psum_outs = [
                        poutp.tile([1, D], F32, tag=f"po{par}", name=f"po{par}")
                        for par in range(2)
                    ]
                    z_pps = [
                        small.tile([128, NCHUNK], F32, tag=f"zpp{par}", name=f"zpp{par}")
                        for par in range(2)
                    ]
                    pends = [[], []]   # awaiting values flush: (ec, vals, c)
                    sq = [[], []]      # awaiting score stage: (feat, vals, c)

                    def flush_pend(par, last):
                        ec_p, vals_p, c_p = pends[par].pop(0)
                        cc_p = c_p % NCC
                        for j in range(NSUB):
                            nc.tensor.matmul(
                                out=psum_outs[par],
                                lhsT=ec_p[:, j : j + 1],
                                rhs=vals_p[:, cc_p * NSUB + j, :],
                                start=(c_p == 0 and j == 0),
                                stop=(last and j == NSUB - 1),
                                skip_group_check=True,
                            )

                    def do_scores(par):
                        # score stage for the OLDEST queued chunk (lag-1: its
                        # tanh has had a full chunk-pair to finish)
                        fe, vt, cp = sq[par].pop(0)
                        scol = scolp.tile([128, NSUB], F32, tag="scol", name="scol")
                        for j in range(NSUB):
                            for ht in range(2):
                                nc.tensor.matmul(
                                    out=scol[:, j : j + 1],
                                    lhsT=fe[:, ht, ts(j, 128)],
                                    rhs=wv_cols[:, ht : ht + 1],
                                    start=(ht == 0),
                                    stop=(ht == 1),
                                )
                        ec = small.tile([128, NSUB], F32R, tag=f"ec{par}", bufs=3)
                        nc.scalar.activation(
                            out=ec,
                            in_=scol,
                            func=AF.Exp,
                            bias=negc[:, 0:1],
                            accum_out=z_pps[par][:, cp : cp + 1],
                        )
                        pends[par].append((ec, vt, cp))

                    for L in range(NL):
                        keys_cur = [kf_hist[(pair, L, 0)], kf_hist[(pair, L, 1)]]
                        vals_cur = [va_hist[(pair, L, 0)], va_hist[(pair, L, 1)]]
                        if L + 1 < NL:
                            issue_dma_for(pair, L + 1)
                        elif pair + 1 < NPAIR:
                            # hoist the NEXT pair's first loads ahead of this
                            # pair's tail so the DMA queue never sits behind
                            # the compute-dependent normalization/stores
                            issue_dma_for(pair + 1, 0)
                        for cc in range(NCC):
                            c = L * NCC + cc

                            # ---- KP for both parities, stationary-major so
                            # each W_k 128x128 slice loads once per chunk.
                            # kp banks interleave accumulation groups, hence
                            # skip_group_check.
                            kps = [
                                [
                                    pkpp.tile(
                                        [128, CHUNK], F32,
                                        tag=f"kp{par}{ht}", name=f"kp{par}{ht}",
                                    )
                                    for ht in range(2)
                                ]
                                for par in range(2)
                            ]
                            for ht in range(2):
                                for dt in range(2):
                                    for par in range(2):
                                        nc.tensor.matmul(
                                            out=kps[par][ht],
                                            lhsT=wk_s[:, dt, ts(ht, 128)],
                                            rhs=keys_cur[par][:, dt, ts(cc, CHUNK)],
                                            start=(dt == 0),
                                            stop=(dt == 1),
                                            skip_group_check=True,
                                        )

                            # ---- per parity: tanh, then lag-2 values flush
                            for par in range(2):
                                feat = mid.tile([128, 2, CHUNK], F16, tag=f"feat{par}")
                                for ht in range(2):
                                    nc.scalar.activation(
                                        out=feat[:, ht, :],
                                        in_=kps[par][ht],
                                        func=AF.Tanh,
                                        bias=q_cols[:, bs[par], ht : ht + 1],
                                        scale=1.0,
                                    )
                                sq[par].append((feat, vals_cur[par], c))
                                if pends[par]:
                                    flush_pend(par, last=False)

                            # ---- S: score columns, exp, Z for chunk c-1
                            for par in range(2):
                                if len(sq[par]) >= 2:
                                    do_scores(par)

                    # ---- tail: drain score + flush stages, normalize, store ----
                    for par in range(2):
                        do_scores(par)
                    for par in range(2):
                        flush_pend(par, last=False)
                        flush_pend(par, last=True)
                    for par in range(2):
                        b = bs[par]
                        # Z = sum over partitions and chunks of z_pp: one
                        # matmul ones^T @ z_pps -> [1,16] column sums on
                        # partition 0, then a free-axis reduce.
                        zrow_ps = scolp.tile([1, NCHUNK], F32, tag="scol", name=f"zr{par}")
                        nc.tensor.matmul(
                            out=zrow_ps, lhsT=ones_col, rhs=z_pps[par]
                        )
                        z1 = small.tile([1, 1], F32, tag=f"z{par}")
                        nc.vector.reduce_sum(
                            out=z1, in_=zrow_ps, axis=mybir.AxisListType.X
                        )
                        rz = small.tile([1, 1], F32, tag=f"rz{par}")
                        nc.vector.reciprocal(out=rz, in_=z1)
                        orow = small.tile([1, D], F32, tag=f"orow{par}")
                        nc.scalar.mul(
                            out=orow, in_=psum_outs[par], mul=rz[0:1, 0:1]
                        )
                        nc.sync.dma_start(out=out_d[b : b + 1, :], in_=orow)

    nc.compile()
    return nc


_NC = None


def _get_nc():
    global _NC
    if _NC is None:
        _NC = build()
    return _NC


def _pack_keys(kcore):
    """[BL, TK, D] f32 -> [NPAIR, NL, 2(par), 128, 2(dt), TT]: the per-load SBUF
    image, with the chip-side t-permutation that matches the values tile layout.

    Values load p-major: va[q, n] holds t = L*TT + q*16 + n with n = cc*4 + j.
    The flush for (cc, j) contracts ec[q, j] against va[:, cc*4+j], so the
    score pipeline must emit chunk-position j*128+q <-> that same t. Scores
    inherit the keys free axis, hence keys position t' = cc*512+j*128+q must
    hold t = q*16 + cc*4 + j:
        kf[p, dt, cc*512 + j*128 + q] = keys[b, L*TT + q*16 + cc*4 + j, dt*128+p]
    """
    a = kcore.reshape(NPAIR, 2, NL, 128, NCC, NSUB, 2, 128)
    # axes: (pair, par, L, q, cc, j, dt, p) -> (pair, L, par, p, dt, cc, j, q)
    a = a.transpose(0, 2, 1, 7, 6, 4, 5, 3)
    return np.ascontiguousarray(a.reshape(NPAIR, NL, 2, 128, 2, TT))


def _pack_vals(vcore):
    """[BL, TK, D] f32 -> [NPAIR, NL, 2(par), 128, NNT, D]: the p-major values
    tile image vv[p, n] = values[2*pair+par, L*TT + p*16 + n]."""
    a = vcore.reshape(NPAIR, 2, NL, 128, NNT, D)
    a = a.transpose(0, 2, 1, 3, 4, 5)
    return np.ascontiguousarray(a)


def make_in_maps(queries, keys, values, W_q, W_k, w_v):
    queries = np.asarray(queries, np.float32)
    keys = np.asarray(keys, np.float32)
    values = np.asarray(values, np.float32)
    W_q = np.ascontiguousarray(np.asarray(W_q, np.float32))
    W_k = np.ascontiguousarray(np.asarray(W_k, np.float32))
    wv2 = np.ascontiguousarray(np.asarray(w_v, np.float32).reshape(1, H))
    in_maps = []
    for i in range(NCORES):
        sl = slice(i * BL, (i + 1) * BL)
        in_maps.append(
            {
                "queries": np.ascontiguousarray(queries[sl]),
                "keys": _pack_keys(keys[sl]),
                "values": _pack_vals(values[sl]),
                "W_q": W_q,
                "W_k": W_k,
                "w_v": wv2,
            }
        )
    return in_maps


def kernel(queries, keys, values, W_q, W_k, w_v):
    nc = _get_nc()
    in_maps = make_in_maps(queries, keys, values, W_q, W_k, w_v)
    res = run_bass_kernel_spmd(nc, in_maps, list(range(NCORES)))
    return np.concatenate([res.results[i]["out"] for i in range(NCORES)], axis=0)
